# revision 14
# baseline (speedup 1.0000x reference)
"""Trainium2 Bass kernel for nn_AttentionSampling (sparse window attention block).

Reference computation (per batch b):
    q = relu(query @ w_q + b_q)                      # [SQ, D],  SQ = SK/F
    k = relu(key   @ w_k + b_k)                      # [SK, D]
    v = value @ w_v + b_v                            # [SK, D]
    w[s, f]  = sum_d q[s, d] * k[4s+f, d]            # windowed dots, F=4
    ao[s, :] = sum_f w[s, f] * v[4s+f, :]
    x  = LN(q + ao);  y = relu(x @ W1 + b1) @ W2 + b2;  out = LN(x + y)

Sharding: 8 cores, each takes 1024 windows (half of one batch) -> 4096 keys.
All windows are independent, so there is no cross-core communication.

Layout strategy ("transposed world"): activations are kept as [d, tokens]
(d on partitions) so every projection runs with the weight matrix stationary
and the activation moving, with zero on-chip transposes of the big tensors.
The host supplies query/key/value pre-transposed per core.  The only PE
transposes are of the small per-block ao accumulator (128x512 per block).
LayerNorms run in the transposed domain using PE ones-matmuls for the
token-wise sums and a PE rank-1 matmul to broadcast mean/rstd across
partitions.
"""

import contextlib

import numpy as np

import concourse.bass as bass
import concourse.bacc as bacc_mod
import concourse.mybir as mybir
import concourse.tile as tile
from concourse.bass import ts, ds
from concourse.bass_utils import run_bass_kernel_spmd

FP32 = mybir.dt.float32
AF = mybir.ActivationFunctionType
OP = mybir.AluOpType

B, SQ, SK, D, F = 4, 2048, 8192, 512, 4
NCORES = 8
WPC = B * SQ // NCORES        # 1024 windows (= queries/tokens) per core
KPC = WPC * F                 # 4096 keys per core
NBLK = WPC // 128             # 8 attention blocks: 128 windows / 512 keys each
DT = D // 128                 # 4 d-tiles
EPS = 1e-5

_CACHE = {}


def _emit_proj_T(nc, pools, w_sb, bias_sb, in_sb, out_sb, out_col0, n):
    """out_sb[:, do, out_col0:+n] = relu(W.T @ in + bias)   (transposed layout).

    in_sb:  [128, DT, n]   moving operand (d_in on partitions)
    w_sb:   [128, DT, 512] stationary tiles (w[d_in, d_out])
    out_sb: [128, DT, ...] (d_out on partitions)
    """
    for do in range(DT):
        ps = pools["psum_big"].tile([128, 512], FP32, tag="proj_ps", name="proj_ps")[:, :n]
        for ki in range(DT):
            nc.tensor.matmul(
                ps,
                lhsT=w_sb[:, ki, ts(do, 128)],
                rhs=in_sb[:, ki, :n],
                start=(ki == 0),
                stop=(ki == DT - 1),
            )
        nc.scalar.activation(
            out=out_sb[:, do, ds(out_col0, n)],
            in_=ps,
            func=AF.Relu,
            bias=bias_sb[:, do : do + 1],
            scale=1.0,
        )


def _emit_ln_T(nc, pools, resid_view, sq_tile, stats_sb, out_cb, n):
    """Transposed-domain LayerNorm over D (partition axis, DT tiles of 128).

    resid_view: [128, DT, n] SBUF residual input (d on partitions)
    sq_tile:    [128, DT, n] scratch for squares
    stats_sb:   [1, 2*n] SBUF scratch (mean | rstd)
    out_cb(dt, t1): callback writing normalized tile t1 [128, n] for d-tile dt
    """
    mean = stats_sb[:, :n]
    rstd = stats_sb[:, n : 2 * n]

    nc.vector.tensor_tensor(sq_tile[:], resid_view, resid_view, op=OP.mult)

    st_sum = pools["psum_stats"].tile([1, 512], FP32, tag="st", name="st_sum")[:, :n]
    for dt in range(DT):
        nc.tensor.matmul(
            st_sum, lhsT=pools["ones_col"], rhs=resid_view[:, dt, :],
            start=(dt == 0), stop=(dt == DT - 1),
        )
    nc.scalar.activation(out=mean, in_=st_sum, func=AF.Copy, scale=1.0 / D)

    st_sq = pools["psum_stats"].tile([1, 512], FP32, tag="st", name="st_sq")[:, :n]
    for dt in range(DT):
        nc.tensor.matmul(
            st_sq, lhsT=pools["ones_col"], rhs=sq_tile[:, dt, :],
            start=(dt == 0), stop=(dt == DT - 1),
        )
    e2 = pools["small"].tile([1, 512], FP32, tag="e2", name="e2")[:, :n]
    var = pools["small"].tile([1, 512], FP32, tag="var", name="var")[:, :n]
    nc.scalar.activation(out=e2, in_=st_sq, func=AF.Copy, scale=1.0 / D)

    # var = E[x^2] - mean^2 ; rstd = 1/sqrt(var + eps)
    nc.vector.tensor_tensor(var, mean, mean, op=OP.mult)
    nc.vector.tensor_tensor(var, e2, var, op=OP.subtract)
    nc.scalar.activation(out=var, in_=var, func=AF.Sqrt, bias=pools["eps_t"], scale=1.0)
    nc.vector.reciprocal(rstd, var)

    # broadcast mean|rstd across partitions via rank-1 matmul
    bc_ps = pools["psum_bcast"].tile([128, 1024], FP32, tag="bc_ps")
    for i in range((2 * n + 511) // 512):
        c0 = i * 512
        cn = min(512, 2 * n - c0)
        nc.tensor.matmul(
            bc_ps[:, ds(c0, cn)], lhsT=pools["ones_row"],
            rhs=stats_sb[:, ds(c0, cn)], start=True, stop=True,
        )
    mean_b = bc_ps[:, :n]
    rstd_b = bc_ps[:, n : 2 * n]
    for dt in range(DT):
        t1 = pools["apply"].tile([128, 512], FP32, tag="ln_t1", name="ln_t1")[:, :n]
        nc.vector.tensor_tensor(t1, resid_view[:, dt, :], mean_b, op=OP.subtract)
        nc.vector.tensor_tensor(t1, t1, rstd_b, op=OP.mult)
        out_cb(dt, t1)


def build_program():
    nc = bacc_mod.Bacc(None, target_bir_lowering=False)

    # ---- DRAM tensors (per-core shapes) ----
    qT_d = nc.dram_tensor("qT", [D, WPC], FP32, kind="ExternalInput")
    kT_d = nc.dram_tensor("kT", [D, KPC], FP32, kind="ExternalInput")
    vT_d = nc.dram_tensor("vT", [D, KPC], FP32, kind="ExternalInput")
    wq_d = nc.dram_tensor("w_q", [D, D], FP32, kind="ExternalInput")
    wk_d = nc.dram_tensor("w_k", [D, D], FP32, kind="ExternalInput")
    wv_d = nc.dram_tensor("w_v", [D, D], FP32, kind="ExternalInput")
    w1_d = nc.dram_tensor("ffn_w1", [D, D], FP32, kind="ExternalInput")
    w2_d = nc.dram_tensor("ffn_w2", [D, D], FP32, kind="ExternalInput")
    bq_d = nc.dram_tensor("b_q", [D], FP32, kind="ExternalInput")
    bk_d = nc.dram_tensor("b_k", [D], FP32, kind="ExternalInput")
    bv_d = nc.dram_tensor("b_v", [D], FP32, kind="ExternalInput")
    b1_d = nc.dram_tensor("ffn_b1", [D], FP32, kind="ExternalInput")
    b2_d = nc.dram_tensor("ffn_b2", [D], FP32, kind="ExternalInput")
    g1_d = nc.dram_tensor("ln1_g", [D], FP32, kind="ExternalInput")
    gb1_d = nc.dram_tensor("ln1_b", [D], FP32, kind="ExternalInput")
    g2_d = nc.dram_tensor("ln2_g", [D], FP32, kind="ExternalInput")
    gb2_d = nc.dram_tensor("ln2_b", [D], FP32, kind="ExternalInput")
    mask_d = nc.dram_tensor("cmask", [128, 512], FP32, kind="ExternalInput")
    ident_d = nc.dram_tensor("cident", [128, 128], FP32, kind="ExternalInput")
    outT_d = nc.dram_tensor("outT", [D, WPC], FP32, kind="ExternalOutput")

    qT_t = qT_d.rearrange("(o p) n -> p o n", p=128)
    kT_t = kT_d.rearrange("(o p) n -> p o n", p=128)
    vT_t = vT_d.rearrange("(o p) n -> p o n", p=128)
    outT_t = outT_d.rearrange("(o p) n -> p o n", p=128)

    with tile.TileContext(nc) as tc, contextlib.ExitStack() as ctx:
        singles = ctx.enter_context(tc.tile_pool(name="singles", bufs=1))
        inp = ctx.enter_context(tc.tile_pool(name="inp", bufs=2))
        ktp_p = ctx.enter_context(tc.tile_pool(name="ktp", bufs=1))
        vfs_p = ctx.enter_context(tc.tile_pool(name="vfs", bufs=2))
        att_p = ctx.enter_context(tc.tile_pool(name="att", bufs=2))
        resid_p = ctx.enter_context(tc.tile_pool(name="resid", bufs=2))
        hT_p = ctx.enter_context(tc.tile_pool(name="hT", bufs=1))
        out_p = ctx.enter_context(tc.tile_pool(name="outp", bufs=2))
        small = ctx.enter_context(tc.tile_pool(name="small", bufs=2))
        apply_p = ctx.enter_context(tc.tile_pool(name="applyp", bufs=2))
        psum_big = ctx.enter_context(tc.tile_pool(name="psb", bufs=3, space="PSUM"))
        psum_small = ctx.enter_context(tc.tile_pool(name="pss", bufs=2, space="PSUM"))
        psum_stats = ctx.enter_context(tc.tile_pool(name="pst", bufs=1, space="PSUM"))
        psum_bcast = ctx.enter_context(tc.tile_pool(name="pbc", bufs=1, space="PSUM"))

        # ---- constants ----
        def load_w(d, tg):
            t = singles.tile([128, DT, 512], FP32, tag=tg)
            nc.sync.dma_start(out=t, in_=d.rearrange("(o p) n -> p o n", p=128))
            return t

        wq_sb = load_w(wq_d, "wq")
        wk_sb = load_w(wk_d, "wk")
        wv_sb = load_w(wv_d, "wv")
        w1_sb = load_w(w1_d, "w1")
        w2_sb = load_w(w2_d, "w2")

        def load_b(d, tg):
            t = singles.tile([128, DT], FP32, tag=tg)
            nc.sync.dma_start(out=t, in_=d.rearrange("(o p) -> p o", p=128))
            return t

        bq_sb = load_b(bq_d, "bq")
        bk_sb = load_b(bk_d, "bk")
        b1_sb = load_b(b1_d, "b1")
        b2_sb = load_b(b2_d, "b2")
        g1_sb = load_b(g1_d, "g1")
        gb1_sb = load_b(gb1_d, "gb1")
        g2_sb = load_b(g2_d, "g2")
        gb2_sb = load_b(gb2_d, "gb2")

        # b_v replicated to all partitions (DMA broadcast: partition step 0)
        bv_rep = singles.tile([128, 512], FP32, tag="bv_rep")
        nc.gpsimd.dma_start(
            out=bv_rep, in_=bass.AP(tensor=bv_d, offset=0, ap=[[0, 128], [1, 512]])
        )

        identity = singles.tile([128, 128], FP32, tag="ident")
        nc.sync.dma_start(out=identity, in_=ident_d[:, :])

        # band mask [128, 512]: 1.0 where 0 <= k - 4p <= 3 else 0
        mask = singles.tile([128, 512], FP32, tag="mask")
        nc.sync.dma_start(out=mask, in_=mask_d[:, :])

        ones_col = singles.tile([128, 1], FP32, tag="ones_col")
        nc.gpsimd.memset(ones_col, 1.0)
        ones_row = singles.tile([1, 128], FP32, tag="ones_row")
        nc.gpsimd.memset(ones_row, 1.0)
        eps_t = singles.tile([1, 1], FP32, tag="eps")
        nc.gpsimd.memset(eps_t, EPS)

        pools = {
            "psum_big": psum_big,
            "psum_stats": psum_stats,
            "psum_bcast": psum_bcast,
            "small": small,
            "apply": apply_p,
            "ones_col": ones_col,
            "ones_row": ones_row,
            "eps_t": eps_t,
        }

        # persistent activations
        qTp = singles.tile([128, DT, WPC], FP32, tag="qTp")
        xT = singles.tile([128, DT, WPC], FP32, tag="xT")

        # ---- phase 1: q projection ----
        for blk in range(WPC // 512):
            q_in = inp.tile([128, DT, 512], FP32, tag="in_t")
            nc.sync.dma_start(out=q_in, in_=qT_t[:, :, ts(blk, 512)])
            _emit_proj_T(nc, pools, wq_sb, bq_sb, q_in, qTp, blk * 512, 512)

        # ---- phase 2: attention blocks (128 windows / 512 keys each) ----
        def emit_front(b):
            """k/v projections + scores + windowed weighting for block b."""
            k_in = inp.tile([128, DT, 512], FP32, tag="in_t")
            nc.sync.dma_start(out=k_in, in_=kT_t[:, :, ts(b, 512)])
            v_in = inp.tile([128, DT, 512], FP32, tag="in_t")
            nc.sync.dma_start(out=v_in, in_=vT_t[:, :, ts(b, 512)])

            kTp = ktp_p.tile([128, DT, 512], FP32, tag="kTp")
            _emit_proj_T(nc, pools, wk_sb, bk_sb, k_in, kTp, 0, 512)

            # v projection, f-strided so psum rows = window index
            vf_sb = vfs_p.tile([128, F, 512], FP32, tag="vf")
            for f in range(F):
                ps = psum_big.tile([128, 512], FP32, tag="proj_ps")
                for ki in range(DT):
                    nc.tensor.matmul(
                        ps,
                        lhsT=v_in[:, ki, f::4],
                        rhs=wv_sb[:, ki, :],
                        start=(ki == 0),
                        stop=(ki == DT - 1),
                    )
                nc.scalar.copy(out=vf_sb[:, f, :], in_=ps)

            # scores: [128 windows, 512 keys] = qTp_blk.T @ kTp
            sc_ps = psum_big.tile([128, 512], FP32, tag="proj_ps")
            for ki in range(DT):
                nc.tensor.matmul(
                    sc_ps,
                    lhsT=qTp[:, ki, ts(b, 128)],
                    rhs=kTp[:, ki, :],
                    start=(ki == 0),
                    stop=(ki == DT - 1),
                )
            # mask + windowed weight extraction
            sm = att_p.tile([128, 512], FP32, tag="sm")
            nc.vector.tensor_tensor(sm, sc_ps, mask, op=OP.mult)
            wts = small.tile([128, F], FP32, tag="wts")
            nc.vector.tensor_reduce(
                out=wts,
                in_=sm.rearrange("p (kw f) -> p f kw", f=F),
                axis=mybir.AxisListType.X,
                op=OP.add,
            )
            # ao[w, :] = b_v + sum_f wts[w, f] * v_f[w, :]
            acc = att_p.tile([128, 512], FP32, tag="ao_acc")
            tmp = att_p.tile([128, 512], FP32, tag="ao_tmp")
            nc.vector.tensor_scalar_mul(acc, vf_sb[:, 0, :], wts[:, 0:1])
            for f in range(1, F):
                nc.vector.tensor_scalar_mul(tmp, vf_sb[:, f, :], wts[:, f : f + 1])
                nc.vector.tensor_tensor(acc, acc, tmp, op=OP.add)
            nc.vector.tensor_tensor(acc, acc, bv_rep, op=OP.add)
            return acc

        def emit_back(b, acc):
            """transpose ao, residual with qT, LN1 -> xT columns for block b."""
            residT = resid_p.tile([128, DT, 128], FP32, tag="residT")
            for dt in range(DT):
                ps_t = psum_small.tile([128, 128], FP32, tag="tr_ps")
                nc.tensor.transpose(ps_t, acc[:, ts(dt, 128)], identity)
                nc.vector.tensor_tensor(
                    residT[:, dt, :], ps_t, qTp[:, dt, ts(b, 128)], op=OP.add
                )
            sq = resid_p.tile([128, DT, 128], FP32, tag="sq1")
            stats = small.tile([1, 256], FP32, tag="stats1")

            def write_x(dt, t1):
                nc.scalar.activation(
                    out=xT[:, dt, ts(b, 128)], in_=t1, func=AF.Identity,
                    bias=gb1_sb[:, dt : dt + 1], scale=g1_sb[:, dt : dt + 1],
                )

            _emit_ln_T(nc, pools, residT[:], sq, stats, write_x, 128)

        prev = None
        for b in range(NBLK):
            acc = emit_front(b)
            if prev is not None:
                emit_back(b - 1, prev)
            prev = acc
        emit_back(NBLK - 1, prev)

        # ---- phase 3: FFN + LN2 per 512-token block ----
        for blk in range(WPC // 512):
            hT = hT_p.tile([128, DT, 512], FP32, tag="hT")
            for ht in range(DT):
                ps = psum_big.tile([128, 512], FP32, tag="proj_ps")
                for ki in range(DT):
                    nc.tensor.matmul(
                        ps,
                        lhsT=w1_sb[:, ki, ts(ht, 128)],
                        rhs=xT[:, ki, ts(blk, 512)],
                        start=(ki == 0),
                        stop=(ki == DT - 1),
                    )
                nc.scalar.activation(
                    out=hT[:, ht, :], in_=ps, func=AF.Relu,
                    bias=b1_sb[:, ht : ht + 1], scale=1.0,
                )
            resid2 = resid_p.tile([128, DT, 512], FP32, tag="resid2")
            for dt in range(DT):
                ps = psum_big.tile([128, 512], FP32, tag="proj_ps")
                for hi in range(DT):
                    nc.tensor.matmul(
                        ps,
                        lhsT=w2_sb[:, hi, ts(dt, 128)],
                        rhs=hT[:, hi, :],
                        start=(hi == 0),
                        stop=(hi == DT - 1),
                    )
                yb = apply_p.tile([128, 512], FP32, tag="yb")
                nc.scalar.activation(
                    out=yb, in_=ps, func=AF.Identity,
                    bias=b2_sb[:, dt : dt + 1], scale=1.0,
                )
                nc.vector.tensor_tensor(
                    resid2[:, dt, :], yb, xT[:, dt, ts(blk, 512)], op=OP.add
                )
            sq2 = hT_p.tile([128, DT, 512], FP32, tag="hT")
            stats2 = small.tile([1, 1024], FP32, tag="stats2")
            out_sb = out_p.tile([128, DT, 512], FP32, tag="out_sb")

            def write_out(dt, t1, out_sb=out_sb):
                nc.scalar.activation(
                    out=out_sb[:, dt, :], in_=t1, func=AF.Identity,
                    bias=gb2_sb[:, dt : dt + 1], scale=g2_sb[:, dt : dt + 1],
                )

            _emit_ln_T(nc, pools, resid2[:], sq2, stats2, write_out, 512)
            nc.sync.dma_start(out=outT_t[:, :, ts(blk, 512)], in_=out_sb)

    nc.finalize()
    return nc


def kernel(**inputs):
    if "prog" not in _CACHE:
        _CACHE["prog"] = build_program()
    nc = _CACHE["prog"]

    f32 = lambda x: np.ascontiguousarray(np.asarray(x), dtype=np.float32)
    query, key_, value = f32(inputs["query"]), f32(inputs["key"]), f32(inputs["value"])

    shared = {
        n: f32(inputs[n])
        for n in ("w_q", "w_k", "w_v", "ffn_w1", "ffn_w2", "b_q", "b_k", "b_v",
                  "ffn_b1", "ffn_b2", "ln1_g", "ln1_b", "ln2_g", "ln2_b")
    }
    p_idx = np.arange(128)[:, None]
    k_idx = np.arange(512)[None, :]
    shared["cmask"] = ((k_idx - 4 * p_idx >= 0) & (k_idx - 4 * p_idx <= 3)).astype(np.float32)
    shared["cident"] = np.eye(128, dtype=np.float32)

    in_maps = []
    for c in range(NCORES):
        bi, half = c // 2, c % 2
        w0 = half * WPC
        m = dict(shared)
        m["qT"] = f32(query[bi, w0 : w0 + WPC, :].T)
        m["kT"] = f32(key_[bi, w0 * F : (w0 + WPC) * F, :].T)
        m["vT"] = f32(value[bi, w0 * F : (w0 + WPC) * F, :].T)
        in_maps.append(m)

    res = run_bass_kernel_spmd(nc, in_maps, core_ids=list(range(NCORES)))
    _CACHE["last_result"] = res
    out = np.empty((B, SQ, D), dtype=np.float32)
    for c in range(NCORES):
        bi, half = c // 2, c % 2
        w0 = half * WPC
        out[bi, w0 : w0 + WPC, :] = res.results[c]["outT"].T
    return out
